# revision 7
# baseline (speedup 1.0000x reference)
"""Bass/Trainium2 kernel for nn_Differential_Attention_60825326846200.

Mathematical reduction of the reference:
  scores[b,h,i,j] = (sum_d q[b,h,i,d] - k[b,h,i,d]) / sqrt(DH) + mask[b,i]
is constant over the key index j, so softmax over j is exactly uniform
(1/S) regardless of q, k, and mask.  Hence
  ctx[b,h,i,:] = mean_j v[b,h,j,:]          (independent of i)
  out[b,i,:]   = (mean_j hidden_b[b,j,:]) @ Wv.T + bv   for every i.
q/k projections and the attention mask cancel exactly.

Distribution across the 8 NeuronCores (single SPMD NEFF, one launch):
  - hidden_b is sharded along the sequence axis: core c reduces its
    [B, S/8, HID] slice to a partial column-sum (scaled by 1/S) on the
    tensor engine, laid out transposed as [128(k), KC*B].
  - a tiny (16 KB) AllReduce combines the partial sums -> every core
    holds mean_j hidden_b (transposed, k on partitions).
  - the output feature dim is sharded: core c holds Wv rows
    [c*128:(c+1)*128], transposes them on the tensor engine (identity
    matmul), computes out_row.T[o,b] = sum_k Wv.T[k,o]*hbar.T[k,b],
    adds the bias, broadcasts the row over the 128-partition axis with
    a rank-1 matmul, and DMA-writes its [B, S, 128] output slice with
    a broadcast (step-0) source access pattern.
Host does data movement only: slicing inputs, concatenating outputs.
"""

import numpy as np

import concourse.bacc as bacc
import concourse.mybir as mybir
import concourse.tile as tile
from concourse.bass_utils import run_bass_kernel_spmd

N_CORES = 8
B, S, HID = 2, 2048, 1024
S_LOC = S // N_CORES  # 256 sequence positions reduced per core
O_LOC = HID // N_CORES  # 128 output features produced per core
KC = HID // 128  # 8 contraction chunks of 128
NT = (B * S_LOC) // 128  # 4 sbuf tiles of hidden_b rows per core
F32 = mybir.dt.float32

_compiled = None


def _body(nc, tc, hb, wv, bv, out, ident_d):
    with (
        tc.tile_pool(name="big", bufs=1) as big,
        tc.tile_pool(name="small", bufs=1) as small,
        tc.tile_pool(name="psum", bufs=1, space="PSUM") as psum,
        tc.tile_pool(name="dram", bufs=1, space="DRAM") as dram,
    ):
        ident = small.tile([128, 128], F32)
        nc.sync.dma_start(ident[:], ident_d[:])
        sones = small.tile([128, 1], F32)
        nc.vector.memset(sones[:], 1.0 / S)
        ones1 = small.tile([1, 128], F32)
        nc.vector.memset(ones1[:], 1.0)
        bv_sb = small.tile([1, O_LOC], F32)
        nc.sync.dma_start(bv_sb[:], bv[:])

        # local column-sum of the hidden_b slice, transposed: [128(k), kc*B+b]
        hb_t = hb.rearrange("(t p) k -> t p k", p=128)
        hb_sbs = []
        for t in range(NT):
            hb_sb = big.tile([128, HID], F32, name=f"hb{t}", tag=f"hb{t}")
            nc.sync.dma_start(hb_sb[:], hb_t[t])
            hb_sbs.append(hb_sb)
        tpb = NT // B  # s-tiles per batch
        psum_hbT = psum.tile([128, KC * B], F32)
        for kc in range(KC):
            for b in range(B):
                col = kc * B + b
                for i in range(tpb):
                    nc.tensor.matmul(
                        psum_hbT[:, col : col + 1],
                        lhsT=hb_sbs[b * tpb + i][:, kc * 128 : (kc + 1) * 128],
                        rhs=sones[:],
                        start=(i == 0),
                        stop=(i == tpb - 1),
                    )
        hbT_loc = small.tile([128, KC * B], F32)
        nc.vector.tensor_copy(hbT_loc[:], psum_hbT[:])

        # AllReduce the 16KB of partial sums -> full (mean_j hidden_b).T
        cc_in = dram.tile([128, KC * B], F32)
        cc_out = dram.tile([128, KC * B], F32, addr_space="Shared")
        nc.sync.dma_start(cc_in[:], hbT_loc[:])
        nc.gpsimd.collective_compute(
            "AllReduce",
            mybir.AluOpType.add,
            replica_groups=[list(range(N_CORES))],
            ins=[cc_in[:]],
            outs=[cc_out[:]],
        )
        hbT = small.tile([128, KC * B], F32)
        nc.sync.dma_start(hbT[:], cc_out[:])

        # transpose this core's Wv rows: wvT[:, kc*128:..] = wv[:, kc*128:..].T
        wv_sb = big.tile([O_LOC, HID], F32)
        nc.sync.dma_start(wv_sb[:], wv[:])
        wvT = big.tile([128, HID], F32)
        for kc in range(KC):
            pt = psum.tile([128, 128], F32, name=f"pt{kc}", tag="pt", bufs=2)
            nc.tensor.transpose(pt[:], wv_sb[:, kc * 128 : (kc + 1) * 128], ident[:])
            nc.vector.tensor_copy(wvT[:, kc * 128 : (kc + 1) * 128], pt[:])

        # out_row[b, o] = sum_k hbar.T[k, b] * Wv.T[k, o], one M=1 chain per
        # batch so every operand stays at base partition 0
        for b in range(B):
            psum_row = psum.tile([1, O_LOC], F32, name=f"prow{b}", tag="prow", bufs=2)
            for kc in range(KC):
                nc.tensor.matmul(
                    psum_row[:],
                    lhsT=hbT[:, kc * B + b : kc * B + b + 1],
                    rhs=wvT[:, kc * 128 : (kc + 1) * 128],
                    start=(kc == 0),
                    stop=(kc == KC - 1),
                )
            row_f = small.tile([1, O_LOC], F32, name=f"rowf{b}")
            nc.vector.tensor_add(row_f[:], psum_row[:], bv_sb[:])

            # broadcast the row across 128 partitions (rank-1 matmul), then
            # DMA the [S, 128] output slice reusing the tile 16x (step-0 AP)
            pbc = psum.tile([128, 128], F32, name=f"pbc{b}", tag="pbc", bufs=2)
            nc.tensor.matmul(
                pbc[:], lhsT=ones1[:], rhs=row_f[:], start=True, stop=True
            )
            bc = big.tile([128, 128], F32, name=f"bc{b}", tag="bc", bufs=2)
            nc.vector.tensor_copy(bc[:], pbc[:])
            dst = out[b].rearrange("(t p) o -> p t o", p=128)
            src = bc[:].unsqueeze(1).broadcast_to([128, S // 128, 128])
            nc.sync.dma_start(dst, src)


def _build():
    nc = bacc.Bacc(
        "TRN2",
        target_bir_lowering=False,
        debug=False,
        enable_asserts=True,
        num_devices=N_CORES,
    )
    hb = nc.dram_tensor("hb", [B * S_LOC, HID], F32, kind="ExternalInput").ap()
    wv = nc.dram_tensor("wv", [O_LOC, HID], F32, kind="ExternalInput").ap()
    bv = nc.dram_tensor("bv", [1, O_LOC], F32, kind="ExternalInput").ap()
    out = nc.dram_tensor("out", [B, S, O_LOC], F32, kind="ExternalOutput").ap()
    ident_d = nc.inline_tensor(np.eye(128, dtype=np.float32), name="ident").ap()

    with tile.TileContext(nc) as tc:
        _body(nc, tc, hb, wv, bv, out, ident_d)
    nc.compile()
    return nc


def get_nc():
    global _compiled
    if _compiled is None:
        _compiled = _build()
    return _compiled


def make_in_maps(inputs):
    hb = np.ascontiguousarray(np.asarray(inputs["hidden_states_b"], dtype=np.float32))
    Wv = np.ascontiguousarray(np.asarray(inputs["Wv"], dtype=np.float32))
    bv = np.ascontiguousarray(np.asarray(inputs["bv"], dtype=np.float32))
    in_maps = []
    for c in range(N_CORES):
        hb_slice = hb[:, c * S_LOC : (c + 1) * S_LOC, :].reshape(B * S_LOC, HID)
        in_maps.append(
            {
                "hb": np.ascontiguousarray(hb_slice),
                "wv": np.ascontiguousarray(Wv[c * O_LOC : (c + 1) * O_LOC, :]),
                "bv": np.ascontiguousarray(
                    bv[c * O_LOC : (c + 1) * O_LOC].reshape(1, O_LOC)
                ),
            }
        )
    return in_maps


def gather_out(results):
    return np.concatenate([results[c]["out"] for c in range(N_CORES)], axis=2)


def kernel(**inputs) -> np.ndarray:
    nc = get_nc()
    res = run_bass_kernel_spmd(nc, make_in_maps(inputs), list(range(N_CORES)))
    return gather_out(res.results)


# revision 9
# speedup vs baseline: 1.6117x; 1.6117x over previous
"""Bass/Trainium2 kernel for nn_Differential_Attention_60825326846200.

Mathematical reduction of the reference:
  scores[b,h,i,j] = (sum_d q[b,h,i,d] - k[b,h,i,d]) / sqrt(DH) + mask[b,i]
is constant over the key index j, so the softmax over j is exactly the
uniform distribution (1/S) regardless of q, k, and the mask.  Hence
  ctx[b,h,i,:] = mean_j v[b,h,j,:]          (independent of i)
  out[b,i,:]   = (mean_j hidden_b[b,j,:]) @ Wv.T + bv   for every i.
The q/k projections and the attention mask cancel exactly.

Distribution across the 8 NeuronCores — two small collective-free SPMD
launches (a cross-core AllReduce costs 40-55us here in barrier+mesh
latency; two extra launch fixed-overheads are cheaper):

  Launch 1 (mean, sequence-sharded): core c reduces its [B, S/8, HID]
  slice of hidden_b to a partial column-sum scaled by 1/S on the tensor
  engine, laid out transposed [128(k), kc*B+b] -> 8KB "part" output.

  Host glue (pure data movement): stack the 8 partial tensors along a
  trailing axis; replicate to all cores.

  Launch 2 (projection, feature-sharded): each core sums the 8 partials
  (one DVE reduce) giving mean_j hidden_b transposed; transposes its
  128 rows of Wv on the tensor engine (identity matmul); computes
  out_row[b, o] = sum_k hbar.T[k,b] * Wv.T[k,o]; adds the bias;
  broadcasts each row across the 128-partition axis with a rank-1
  matmul; DMA-writes its [B, S, 128] output slice with a broadcast
  (step-0) source access pattern.

Host does data movement only: slicing/stacking/concatenation.
"""

import numpy as np

import concourse.bacc as bacc
import concourse.mybir as mybir
import concourse.tile as tile
from concourse.bass_utils import run_bass_kernel_spmd

N_CORES = 8
B, S, HID = 2, 2048, 1024
S_LOC = S // N_CORES  # 256 sequence positions reduced per core
O_LOC = HID // N_CORES  # 128 output features produced per core
KC = HID // 128  # 8 contraction chunks of 128
F32 = mybir.dt.float32

_compiled = None


def _new_nc():
    return bacc.Bacc(
        "TRN2",
        target_bir_lowering=False,
        debug=False,
        enable_asserts=True,
        num_devices=N_CORES,
    )


def _build_mean():
    """Launch 1: partial (1/S)-scaled column-sum of this core's hidden_b
    slice, transposed so k lands on partitions: part[k % 128, kc*B + b]."""
    nc = _new_nc()
    hb = nc.dram_tensor("hb", [B * S_LOC, HID], F32, kind="ExternalInput").ap()
    part = nc.dram_tensor("part", [128, KC * B], F32, kind="ExternalOutput").ap()

    with tile.TileContext(nc) as tc:
        with (
            tc.tile_pool(name="big", bufs=1) as big,
            tc.tile_pool(name="small", bufs=1) as small,
            tc.tile_pool(name="psum", bufs=1, space="PSUM") as psum,
        ):
            sones = small.tile([128, 1], F32)
            nc.vector.memset(sones[:], 1.0 / S)

            # one [128, 2*HID] tile per batch (s on partitions, (t, k) on
            # free), loaded by one DMA each on separate HWDGE rings
            hb_b = hb.rearrange("(b t p) k -> b p t k", b=B, p=128)
            tiles = []
            for b in range(B):
                t_sb = big.tile([128, 2 * HID], F32, name=f"hbsb{b}")
                eng = nc.sync if b == 0 else nc.scalar
                eng.dma_start(t_sb[:].rearrange("p (t k) -> p t k", t=2), hb_b[b])
                tiles.append(t_sb)

            psum_hbT = psum.tile([128, KC * B], F32)
            for kc in range(KC):
                for b in range(B):
                    col = kc * B + b
                    for t in range(2):
                        nc.tensor.matmul(
                            psum_hbT[:, col : col + 1],
                            lhsT=tiles[b][:, t * HID + kc * 128 : t * HID + (kc + 1) * 128],
                            rhs=sones[:],
                            start=(t == 0),
                            stop=(t == 1),
                        )
            part_sb = small.tile([128, KC * B], F32)
            nc.vector.tensor_copy(part_sb[:], psum_hbT[:])
            nc.sync.dma_start(part[:], part_sb[:])
    nc.compile()
    return nc


def _build_proj():
    """Launch 2: sum the 8 partials, project through this core's Wv rows,
    add bias, broadcast over the sequence axis, write the output slice."""
    nc = _new_nc()
    parts = nc.dram_tensor("parts", [128, KC * B, N_CORES], F32, kind="ExternalInput").ap()
    wv = nc.dram_tensor("wv", [O_LOC, HID], F32, kind="ExternalInput").ap()
    bv = nc.dram_tensor("bv", [1, O_LOC], F32, kind="ExternalInput").ap()
    out = nc.dram_tensor("out", [B, S, O_LOC], F32, kind="ExternalOutput").ap()
    ident_d = nc.inline_tensor(np.eye(128, dtype=np.float32), name="ident").ap()

    with tile.TileContext(nc) as tc:
        with (
            tc.tile_pool(name="big", bufs=1) as big,
            tc.tile_pool(name="small", bufs=1) as small,
            tc.tile_pool(name="psum", bufs=1, space="PSUM") as psum,
        ):
            ident = small.tile([128, 128], F32)
            nc.sync.dma_start(ident[:], ident_d[:])
            ones1 = small.tile([1, 128], F32)
            nc.vector.memset(ones1[:], 1.0)
            bv_sb = small.tile([1, O_LOC], F32)
            nc.sync.dma_start(bv_sb[:], bv[:])

            parts_sb = small.tile([128, KC * B * N_CORES], F32)
            nc.sync.dma_start(parts_sb[:], parts[:])
            hbT = small.tile([128, KC * B], F32)
            nc.vector.reduce_sum(
                hbT[:],
                parts_sb[:].rearrange("p (c n) -> p c n", n=N_CORES),
                axis=mybir.AxisListType.X,
            )

            # wvT[:, kc*128:..] = wv[:, kc*128:..].T via identity matmuls
            wv_sb = big.tile([O_LOC, HID], F32)
            nc.scalar.dma_start(wv_sb[:], wv[:])
            wvT = big.tile([128, HID], F32)
            for kc in range(KC):
                pt = psum.tile([128, 128], F32, name=f"pt{kc}", tag="pt", bufs=2)
                nc.tensor.transpose(pt[:], wv_sb[:, kc * 128 : (kc + 1) * 128], ident[:])
                nc.vector.tensor_copy(wvT[:, kc * 128 : (kc + 1) * 128], pt[:])

            # out_row[b, o] = sum_k hbar.T[k, b] * Wv.T[k, o]  (M=1 chains)
            for b in range(B):
                psum_row = psum.tile([1, O_LOC], F32, name=f"prow{b}", tag="prow", bufs=2)
                for kc in range(KC):
                    nc.tensor.matmul(
                        psum_row[:],
                        lhsT=hbT[:, kc * B + b : kc * B + b + 1],
                        rhs=wvT[:, kc * 128 : (kc + 1) * 128],
                        start=(kc == 0),
                        stop=(kc == KC - 1),
                    )
                row_f = small.tile([1, O_LOC], F32, name=f"rowf{b}")
                nc.vector.tensor_add(row_f[:], psum_row[:], bv_sb[:])

                # broadcast across partitions (rank-1 matmul), then one DMA
                # per batch reusing the 128-row tile 16x (step-0 source AP)
                pbc = psum.tile([128, 128], F32, name=f"pbc{b}", tag="pbc", bufs=2)
                nc.tensor.matmul(
                    pbc[:], lhsT=ones1[:], rhs=row_f[:], start=True, stop=True
                )
                bc = big.tile([128, 128], F32, name=f"bc{b}", tag="bc", bufs=2)
                nc.vector.tensor_copy(bc[:], pbc[:])
                dst = out[b].rearrange("(t p) o -> p t o", p=128)
                src = bc[:].unsqueeze(1).broadcast_to([128, S // 128, 128])
                eng = nc.sync if b == 0 else nc.scalar
                eng.dma_start(dst, src)
    nc.compile()
    return nc


def get_ncs():
    global _compiled
    if _compiled is None:
        _compiled = (_build_mean(), _build_proj())
    return _compiled


def make_mean_in_maps(inputs):
    hb = np.ascontiguousarray(np.asarray(inputs["hidden_states_b"], dtype=np.float32))
    return [
        {
            "hb": np.ascontiguousarray(
                hb[:, c * S_LOC : (c + 1) * S_LOC, :].reshape(B * S_LOC, HID)
            )
        }
        for c in range(N_CORES)
    ]


def make_proj_in_maps(inputs, part_results):
    Wv = np.ascontiguousarray(np.asarray(inputs["Wv"], dtype=np.float32))
    bv = np.ascontiguousarray(np.asarray(inputs["bv"], dtype=np.float32))
    parts = np.ascontiguousarray(
        np.stack([part_results[c]["part"] for c in range(N_CORES)], axis=-1)
    )
    return [
        {
            "parts": parts,
            "wv": np.ascontiguousarray(Wv[c * O_LOC : (c + 1) * O_LOC, :]),
            "bv": np.ascontiguousarray(
                bv[c * O_LOC : (c + 1) * O_LOC].reshape(1, O_LOC)
            ),
        }
        for c in range(N_CORES)
    ]


def gather_out(results):
    return np.concatenate([results[c]["out"] for c in range(N_CORES)], axis=2)


def kernel(**inputs) -> np.ndarray:
    nc_mean, nc_proj = get_ncs()
    cores = list(range(N_CORES))
    res1 = run_bass_kernel_spmd(nc_mean, make_mean_in_maps(inputs), cores)
    res2 = run_bass_kernel_spmd(
        nc_proj, make_proj_in_maps(inputs, res1.results), cores
    )
    return gather_out(res2.results)


# revision 10
# speedup vs baseline: 1.8882x; 1.1716x over previous
"""Bass/Trainium2 kernel for nn_Differential_Attention_60825326846200.

Mathematical reduction of the reference:
  scores[b,h,i,j] = (sum_d q[b,h,i,d] - k[b,h,i,d]) / sqrt(DH) + mask[b,i]
is constant over the key index j, so the softmax over j is exactly the
uniform distribution (1/S) regardless of q, k, and the mask.  Hence
  ctx[b,h,i,:] = mean_j v[b,h,j,:]          (independent of i)
  out[b,i,:]   = (mean_j hidden_b[b,j,:]) @ Wv.T + bv   for every i.
The q/k projections and the attention mask cancel exactly.

Distribution across the 8 NeuronCores — two small collective-free SPMD
launches (a cross-core AllReduce costs 40-55us here in barrier+mesh
latency; two extra launch fixed-overheads are cheaper).  Shards are laid
out on the host in transposed, partition-tiled form (pure permutation,
no host arithmetic) so each launch needs no on-device transposes:

  Launch 1 (mean, sequence-sharded): core c gets its [B, S/8, HID]
  slice of hidden_b as [128(p), KC(kc), B, S/8] (k = kc*128+p on the
  partition axis) and reduces the trailing sequence axis with two DVE
  reduce_sum instructions -> 8KB partial-sum output "part" [128, KC*B].

  Host glue (pure data movement): stack the 8 partial tensors along a
  trailing axis; replicate to all cores.

  Launch 2 (projection, feature-sharded): each core sums the 8 partials
  (one DVE reduce), scales by 1/S, computes
  out_row[b, o] = sum_k hbar.T[k,b] * Wv.T[k,o] with 16 M=1 tensor-
  engine matmuls against its host-pre-transposed Wv shard, adds the
  bias, broadcasts both rows across the 128-partition axis with one
  rank-1 matmul, and DMA-writes its [B, S, 128] output slice with
  broadcast (step-0) source access patterns.

Host does data movement only: slicing/permutation/concatenation.
"""

import numpy as np

import concourse.bacc as bacc
import concourse.mybir as mybir
import concourse.tile as tile
from concourse.bass_utils import run_bass_kernel_spmd

N_CORES = 8
B, S, HID = 2, 2048, 1024
S_LOC = S // N_CORES  # 256 sequence positions reduced per core
O_LOC = HID // N_CORES  # 128 output features produced per core
KC = HID // 128  # 8 contraction chunks of 128
F32 = mybir.dt.float32

_compiled = None


def _new_nc():
    return bacc.Bacc(
        "TRN2",
        target_bir_lowering=False,
        debug=False,
        enable_asserts=True,
        num_devices=N_CORES,
    )


def _build_mean():
    """Launch 1: partial column-sum of this core's hidden_b slice.
    Input "hbt" [128, KC, B, S_LOC]: hbt[p, kc, b, s] = hb[b, s, kc*128+p].
    Output "part" [128, KC*B] with column kc*B + b (raw sums, unscaled)."""
    nc = _new_nc()
    hbt = nc.dram_tensor("hbt", [128, KC, B, S_LOC], F32, kind="ExternalInput").ap()
    part = nc.dram_tensor("part", [128, KC * B], F32, kind="ExternalOutput").ap()

    half = KC // 2 * B * S_LOC  # free elements per ring-half
    with tile.TileContext(nc) as tc:
        with (
            tc.tile_pool(name="big", bufs=1) as big,
            tc.tile_pool(name="small", bufs=1) as small,
        ):
            part_sb = small.tile([128, KC * B], F32)
            for h, eng in enumerate((nc.sync, nc.scalar)):
                t_sb = big.tile([128, half], F32, name=f"hbsb{h}")
                eng.dma_start(
                    t_sb[:].rearrange("p (kc b s) -> p kc b s", kc=KC // 2, b=B),
                    hbt[:, h * (KC // 2) : (h + 1) * (KC // 2)],
                )
                nc.vector.reduce_sum(
                    part_sb[:, h * (KC // 2) * B : (h + 1) * (KC // 2) * B],
                    t_sb[:].rearrange("p (kc b s) -> p kc b s", kc=KC // 2, b=B),
                    axis=mybir.AxisListType.X,
                )
            nc.sync.dma_start(part[:], part_sb[:])
    nc.compile()
    return nc


def _build_proj():
    """Launch 2: sum the 8 partials, scale by 1/S, project through this
    core's (host-pre-transposed) Wv rows, add bias, broadcast over the
    sequence axis, write the [B, S, O_LOC] output slice.
    Input "wvt" [128, KC, O_LOC]: wvt[p, kc, o] = Wv[c*128+o, kc*128+p]."""
    nc = _new_nc()
    parts = nc.dram_tensor(
        "parts", [128, KC * B, N_CORES], F32, kind="ExternalInput"
    ).ap()
    wvt = nc.dram_tensor("wvt", [128, KC, O_LOC], F32, kind="ExternalInput").ap()
    bv = nc.dram_tensor("bv", [1, O_LOC], F32, kind="ExternalInput").ap()
    out = nc.dram_tensor("out", [B, S, O_LOC], F32, kind="ExternalOutput").ap()

    with tile.TileContext(nc) as tc:
        with (
            tc.tile_pool(name="big", bufs=1) as big,
            tc.tile_pool(name="small", bufs=1) as small,
            tc.tile_pool(name="psum", bufs=1, space="PSUM") as psum,
        ):
            ones1 = small.tile([1, 128], F32)
            nc.vector.memset(ones1[:], 1.0)
            bv_sb = small.tile([1, O_LOC], F32)
            nc.sync.dma_start(bv_sb[:], bv[:])

            wvT = big.tile([128, KC * O_LOC], F32)
            nc.scalar.dma_start(
                wvT[:].rearrange("p (kc o) -> p kc o", kc=KC), wvt[:]
            )

            parts_sb = small.tile([128, KC * B * N_CORES], F32)
            nc.sync.dma_start(parts_sb[:], parts[:])
            hbT = small.tile([128, KC * B], F32)
            nc.vector.reduce_sum(
                hbT[:],
                parts_sb[:].rearrange("p (c n) -> p c n", n=N_CORES),
                axis=mybir.AxisListType.X,
            )
            nc.vector.tensor_scalar_mul(hbT[:], hbT[:], 1.0 / S)

            # out_row[b, o] = sum_k hbar.T[k, b] * Wv.T[k, o]  (M=1 chains);
            # both rows land side by side in one [1, 256] tile
            row_f = small.tile([1, B * O_LOC], F32)
            for b in range(B):
                psum_row = psum.tile(
                    [1, O_LOC], F32, name=f"prow{b}", tag="prow", bufs=2
                )
                for kc in range(KC):
                    nc.tensor.matmul(
                        psum_row[:],
                        lhsT=hbT[:, kc * B + b : kc * B + b + 1],
                        rhs=wvT[:, kc * O_LOC : (kc + 1) * O_LOC],
                        start=(kc == 0),
                        stop=(kc == KC - 1),
                    )
                nc.vector.tensor_add(
                    row_f[:, b * O_LOC : (b + 1) * O_LOC], psum_row[:], bv_sb[:]
                )

            # one rank-1 matmul broadcasts both rows across 128 partitions
            pbc = psum.tile([128, B * O_LOC], F32)
            nc.tensor.matmul(pbc[:], lhsT=ones1[:], rhs=row_f[:], start=True, stop=True)
            bc = big.tile([128, B * O_LOC], F32)
            nc.vector.tensor_copy(bc[:], pbc[:])

            # write each batch's [S, O_LOC] slice reusing the 128-row tile
            # 16x (step-0 source AP); separate engines -> parallel rings
            for b in range(B):
                dst = out[b].rearrange("(t p) o -> p t o", p=128)
                src = (
                    bc[:, b * O_LOC : (b + 1) * O_LOC]
                    .unsqueeze(1)
                    .broadcast_to([128, S // 128, O_LOC])
                )
                eng = nc.sync if b == 0 else nc.scalar
                eng.dma_start(dst, src)
    nc.compile()
    return nc


def get_ncs():
    global _compiled
    if _compiled is None:
        _compiled = (_build_mean(), _build_proj())
    return _compiled


def make_mean_in_maps(inputs):
    hb = np.asarray(inputs["hidden_states_b"], dtype=np.float32)
    # [B, S, HID] -> per core [128, KC, B, S_LOC] (pure permutation)
    maps = []
    for c in range(N_CORES):
        sl = hb[:, c * S_LOC : (c + 1) * S_LOC, :]  # [B, S_LOC, HID]
        t = sl.reshape(B, S_LOC, KC, 128).transpose(3, 2, 0, 1)
        maps.append({"hbt": np.ascontiguousarray(t)})
    return maps


def make_proj_in_maps(inputs, part_results):
    Wv = np.asarray(inputs["Wv"], dtype=np.float32)
    bv = np.asarray(inputs["bv"], dtype=np.float32)
    parts = np.ascontiguousarray(
        np.stack([part_results[c]["part"] for c in range(N_CORES)], axis=-1)
    )
    maps = []
    for c in range(N_CORES):
        w = Wv[c * O_LOC : (c + 1) * O_LOC, :]  # [O_LOC, HID]
        wt = w.reshape(O_LOC, KC, 128).transpose(2, 1, 0)  # [128, KC, O_LOC]
        maps.append(
            {
                "parts": parts,
                "wvt": np.ascontiguousarray(wt),
                "bv": np.ascontiguousarray(
                    bv[c * O_LOC : (c + 1) * O_LOC].reshape(1, O_LOC)
                ),
            }
        )
    return maps


def gather_out(results):
    return np.concatenate([results[c]["out"] for c in range(N_CORES)], axis=2)


def kernel(**inputs) -> np.ndarray:
    nc_mean, nc_proj = get_ncs()
    cores = list(range(N_CORES))
    res1 = run_bass_kernel_spmd(nc_mean, make_mean_in_maps(inputs), cores)
    res2 = run_bass_kernel_spmd(nc_proj, make_proj_in_maps(inputs, res1.results), cores)
    return gather_out(res2.results)


# revision 12
# speedup vs baseline: 2.0002x; 1.0593x over previous
"""Bass/Trainium2 kernel for nn_Differential_Attention_60825326846200.

Mathematical reduction of the reference:
  scores[b,h,i,j] = (sum_d q[b,h,i,d] - k[b,h,i,d]) / sqrt(DH) + mask[b,i]
is constant over the key index j, so the softmax over j is exactly the
uniform distribution (1/S) regardless of q, k, and the mask.  Hence
  ctx[b,h,i,:] = mean_j v[b,h,j,:]          (independent of i)
  out[b,i,:]   = (mean_j hidden_b[b,j,:]) @ Wv.T + bv   for every i.
The q/k projections and the attention mask cancel exactly.

Distribution across the 8 NeuronCores — two small collective-free SPMD
launches (a cross-core AllReduce costs 40-55us here in barrier+mesh
latency; two extra launch fixed-overheads are cheaper).  Shards are laid
out on the host in transposed, partition-tiled form (pure permutation,
no host arithmetic) so each launch needs no on-device transposes:

  Launch 1 (mean, sequence-sharded): core c gets its [B, S/8, HID]
  slice of hidden_b as [128(p), KC(kc), B, S/8] (k = kc*128+p on the
  partition axis) and reduces the trailing sequence axis with two DVE
  reduce_sum instructions -> 8KB partial-sum output "part" [128, KC*B].

  Host glue (pure data movement): stack the 8 partial tensors along a
  trailing axis; replicate to all cores.

  Launch 2 (projection, feature-sharded): each core sums the 8 partials
  (one DVE reduce), scales by 1/S, computes
  out_row[b, o] = sum_k hbar.T[k,b] * Wv.T[k,o] with 16 M=1 tensor-
  engine matmuls against its host-pre-transposed Wv shard, adds the
  bias, broadcasts both rows across the 128-partition axis with one
  rank-1 matmul, and DMA-writes its [B, S, 128] output slice with
  broadcast (step-0) source access patterns.

Host does data movement only: slicing/permutation/concatenation.
"""

import numpy as np

import concourse.bacc as bacc
import concourse.mybir as mybir
import concourse.tile as tile
from concourse.bass_utils import run_bass_kernel_spmd

N_CORES = 8
B, S, HID = 2, 2048, 1024
S_LOC = S // N_CORES  # 256 sequence positions reduced per core
O_LOC = HID // N_CORES  # 128 output features produced per core
KC = HID // 128  # 8 contraction chunks of 128
F32 = mybir.dt.float32

_compiled = None


def _new_nc():
    return bacc.Bacc(
        "TRN2",
        target_bir_lowering=False,
        debug=False,
        enable_asserts=True,
        num_devices=N_CORES,
    )


def _build_mean():
    """Launch 1: partial column-sum of this core's hidden_b slice.
    Input "hbt" [128, KC, B, S_LOC]: hbt[p, kc, b, s] = hb[b, s, kc*128+p].
    Output "part" [128, KC*B] with column kc*B + b (raw sums, unscaled)."""
    nc = _new_nc()
    hbt = nc.dram_tensor("hbt", [128, KC, B, S_LOC], F32, kind="ExternalInput").ap()
    part = nc.dram_tensor("part", [128, KC * B], F32, kind="ExternalOutput").ap()

    nch = 4  # DMA/reduce pipeline chunks (kc pairs), alternating HWDGE rings
    kpc = KC // nch  # kc per chunk
    chunk = kpc * B * S_LOC  # free elements per chunk
    with tile.TileContext(nc) as tc:
        with (
            tc.tile_pool(name="big", bufs=1) as big,
            tc.tile_pool(name="small", bufs=1) as small,
        ):
            part_sb = small.tile([128, KC * B], F32)
            for h in range(nch):
                eng = nc.sync if h % 2 == 0 else nc.scalar
                t_sb = big.tile([128, chunk], F32, name=f"hbsb{h}")
                eng.dma_start(
                    t_sb[:].rearrange("p (kc b s) -> p kc b s", kc=kpc, b=B),
                    hbt[:, h * kpc : (h + 1) * kpc],
                )
                nc.vector.reduce_sum(
                    part_sb[:, h * kpc * B : (h + 1) * kpc * B],
                    t_sb[:].rearrange("p (kc b s) -> p kc b s", kc=kpc, b=B),
                    axis=mybir.AxisListType.X,
                )
            nc.sync.dma_start(part[:], part_sb[:])
    nc.compile()
    return nc


def _build_proj():
    """Launch 2: sum the 8 partials, scale by 1/S, project through this
    core's (host-pre-transposed) Wv rows, add bias, broadcast over the
    sequence axis, write the [B, S, O_LOC] output slice.
    Input "wvt" [128, KC, O_LOC]: wvt[p, kc, o] = Wv[c*128+o, kc*128+p]."""
    nc = _new_nc()
    parts = nc.dram_tensor(
        "parts", [128, KC * B, N_CORES], F32, kind="ExternalInput"
    ).ap()
    wvt = nc.dram_tensor("wvt", [128, KC, O_LOC], F32, kind="ExternalInput").ap()
    bv = nc.dram_tensor("bv", [1, O_LOC], F32, kind="ExternalInput").ap()
    out = nc.dram_tensor("out", [B, S, O_LOC], F32, kind="ExternalOutput").ap()

    with tile.TileContext(nc) as tc:
        with (
            tc.tile_pool(name="big", bufs=1) as big,
            tc.tile_pool(name="small", bufs=1) as small,
            tc.tile_pool(name="psum", bufs=1, space="PSUM") as psum,
        ):
            ones1 = small.tile([1, 128], F32)
            nc.vector.memset(ones1[:], 1.0)
            bv_sb = small.tile([1, O_LOC], F32)
            nc.sync.dma_start(bv_sb[:], bv[:])

            wvT = big.tile([128, KC * O_LOC], F32)
            nc.scalar.dma_start(
                wvT[:].rearrange("p (kc o) -> p kc o", kc=KC), wvt[:]
            )

            parts_sb = small.tile([128, KC * B * N_CORES], F32)
            nc.sync.dma_start(parts_sb[:], parts[:])
            hbT = small.tile([128, KC * B], F32)
            nc.vector.reduce_sum(
                hbT[:],
                parts_sb[:].rearrange("p (c n) -> p c n", n=N_CORES),
                axis=mybir.AxisListType.X,
            )
            nc.vector.tensor_scalar_mul(hbT[:], hbT[:], 1.0 / S)

            # fused projection + partition-broadcast + bias, all on the
            # tensor engine: the stationary operand is the hbar.T column
            # broadcast (step-0) across all 128 output partitions, so
            #   pbc[p, o] = sum_k hbar.T[k,b] * Wv.T[k,o]   for every p,
            # then one rank-1 accumulating matmul adds the bias row
            pbc = psum.tile([128, B * O_LOC], F32)
            bc = big.tile([128, B * O_LOC], F32)
            tsplit = [0, 5, 10, S // 128]  # output thirds per engine
            engs = (nc.sync, nc.scalar, nc.gpsimd)
            for b in range(B):
                pb = pbc[:, b * O_LOC : (b + 1) * O_LOC]
                for kc in range(KC):
                    nc.tensor.matmul(
                        pb,
                        lhsT=hbT[:, kc * B + b : kc * B + b + 1].broadcast_to(
                            [128, 128]
                        ),
                        rhs=wvT[:, kc * O_LOC : (kc + 1) * O_LOC],
                        start=(kc == 0),
                        stop=False,
                    )
                nc.tensor.matmul(
                    pb, lhsT=ones1[:], rhs=bv_sb[:], start=False, stop=True
                )
                bcb = bc[:, b * O_LOC : (b + 1) * O_LOC]
                nc.vector.tensor_copy(bcb, pb)

                # write this batch's [S, O_LOC] slice reusing the 128-row
                # tile 16x (step-0 source AP), split across 3 DMA paths
                dst = out[b].rearrange("(t p) o -> p t o", p=128)
                for eng, lo, hi in zip(engs, tsplit, tsplit[1:]):
                    src = bcb.unsqueeze(1).broadcast_to([128, hi - lo, O_LOC])
                    eng.dma_start(dst[:, lo:hi], src)
    nc.compile()
    return nc


def get_ncs():
    global _compiled
    if _compiled is None:
        _compiled = (_build_mean(), _build_proj())
    return _compiled


def make_mean_in_maps(inputs):
    hb = np.asarray(inputs["hidden_states_b"], dtype=np.float32)
    # [B, S, HID] -> per core [128, KC, B, S_LOC] (pure permutation)
    maps = []
    for c in range(N_CORES):
        sl = hb[:, c * S_LOC : (c + 1) * S_LOC, :]  # [B, S_LOC, HID]
        t = sl.reshape(B, S_LOC, KC, 128).transpose(3, 2, 0, 1)
        maps.append({"hbt": np.ascontiguousarray(t)})
    return maps


def make_proj_in_maps(inputs, part_results):
    Wv = np.asarray(inputs["Wv"], dtype=np.float32)
    bv = np.asarray(inputs["bv"], dtype=np.float32)
    parts = np.ascontiguousarray(
        np.stack([part_results[c]["part"] for c in range(N_CORES)], axis=-1)
    )
    maps = []
    for c in range(N_CORES):
        w = Wv[c * O_LOC : (c + 1) * O_LOC, :]  # [O_LOC, HID]
        wt = w.reshape(O_LOC, KC, 128).transpose(2, 1, 0)  # [128, KC, O_LOC]
        maps.append(
            {
                "parts": parts,
                "wvt": np.ascontiguousarray(wt),
                "bv": np.ascontiguousarray(
                    bv[c * O_LOC : (c + 1) * O_LOC].reshape(1, O_LOC)
                ),
            }
        )
    return maps


def gather_out(results):
    return np.concatenate([results[c]["out"] for c in range(N_CORES)], axis=2)


def kernel(**inputs) -> np.ndarray:
    nc_mean, nc_proj = get_ncs()
    cores = list(range(N_CORES))
    res1 = run_bass_kernel_spmd(nc_mean, make_mean_in_maps(inputs), cores)
    res2 = run_bass_kernel_spmd(nc_proj, make_proj_in_maps(inputs, res1.results), cores)
    return gather_out(res2.results)
